# revision 1
# baseline (speedup 1.0000x reference)
"""Trainium2 Bass kernel for nn_BERTEmbedding (fused per-index affine + sinusoidal PE).

Math (per batch b, vocab-position v, embed index e):
    out[b,v,e] = s0[b,v]*flux_w[v,e] + flux_b[v,e]
               + s2[b,v]*time_w[v,e] + time_b[v,e]
               + (e even: sin(s1[b,v]*div[e/2]) ; e odd: cos(s1[b,v]*div[(e-1)/2]))

Sharding: vocab axis V=4096 split across 8 cores (512 rows each); every core
handles all 16 batches of its vocab shard.  The weight tables are sharded with
the vocab axis, so each core only ever reads its 512-row slices.

Device strategy (per core, 4 v-tiles x 16 batches = 64 work items of [128,768]):
  - TensorE: psum = diag(s0) @ fw + diag(s2) @ tw + I @ bsum   (float32r matmuls,
    diagonal-scaling trick; bsum = flux_b + time_b folded on host)
  - ScalarE: pe tile via Sin activation, laid out as [sin half | cos half] so
    every ACT write is contiguous.  ScalarE's Sin is only valid on [-pi, pi]:
      k >= KLO: |s1|*d_KLO + pi/2 < pi for this problem's inputs -> direct Sin
                with per-partition scale=s1
      k <  KLO: host ships integer phase codes combo_n[b,v,:] (bf16-exact):
                arg*(2/pi) = s1*dv2' + combo_n with dv2' = d_k*2/pi, where
                combo_n = j - 4*rint((s1*d_k + j*pi/2)/2pi), j in {0,1}.
                One fused DVE op builds r' and one Sin(scale=pi/2) evaluates it.
  - VectorE: builds diag tiles (tensor_scalar on identity), the lo-lane fused
    angle op, and the single merge out = psum + pe (interleaving sin/cos via
    the read access pattern; also evacuates PSUM)
  - DMA: table + combo loads once per v-tile; one 393KB store per work item
"""

import math

import numpy as np

try:
    import concourse.bass as bass
except ImportError:  # harness containers keep the repo at /opt/trn_rl_repo
    import sys

    sys.path.insert(0, "/opt/trn_rl_repo")
    import concourse.bass as bass

import concourse.bacc as bacc
import concourse.tile as tile
from concourse import mybir
from concourse.bass_utils import run_bass_kernel_spmd

B, V, E = 16, 4096, 768
EH = E // 2  # 384 angle lanes
KLO = 48  # angle lanes fixed up via the host combo tensor
N_CORES = 8
V_SHARD = V // N_CORES  # 512
VT = V_SHARD // 128  # 4 v-tiles per core
F32 = mybir.dt.float32
F32R = mybir.dt.float32r
BF16 = mybir.dt.bfloat16

TWO_PI = 2.0 * math.pi
HALF_PI = float(np.float32(math.pi / 2.0))
# keep reduced angles strictly inside ScalarE's [-pi, pi] spline domain
SIN_SAFETY = 1.0 - 1e-6
# direct-Sin lanes need |s1|*d_KLO + pi/2 <= pi
S1_LIMIT = (math.pi / 2.0) / math.exp(-KLO * math.log(10000.0) / EH)


def build_bass() -> "bass.Bass":
    from contextlib import ExitStack

    nc = bacc.Bacc(
        "TRN2",
        target_bir_lowering=False,
        debug=False,
        num_devices=N_CORES,
    )
    Alu = mybir.AluOpType

    seq_d = nc.dram_tensor("seq", [128, VT * B * 3], F32, kind="ExternalInput")
    fw_d = nc.dram_tensor("fw", [V_SHARD, E], F32R, kind="ExternalInput")
    tw_d = nc.dram_tensor("tw", [V_SHARD, E], F32R, kind="ExternalInput")
    bs_d = nc.dram_tensor("bs", [V_SHARD, E], F32R, kind="ExternalInput")
    dv_d = nc.dram_tensor("dv", [128, EH], F32, kind="ExternalInput")
    dv2_d = nc.dram_tensor("dv2lo", [128, 2 * KLO], F32, kind="ExternalInput")
    cmb_d = nc.dram_tensor("combo", [128, VT * B * 2 * KLO], BF16, kind="ExternalInput")
    eye_d = nc.dram_tensor("eye", [128, 128], F32R, kind="ExternalInput")
    out_d = nc.dram_tensor("out", [B, V_SHARD, E], F32, kind="ExternalOutput")

    with tile.TileContext(nc) as tc, ExitStack() as ctx:
        const_pool = ctx.enter_context(tc.tile_pool(name="const", bufs=1))
        tab_pool = ctx.enter_context(tc.tile_pool(name="tables", bufs=2))
        diag_pool = ctx.enter_context(tc.tile_pool(name="diag", bufs=6))
        ang_pool = ctx.enter_context(tc.tile_pool(name="ang", bufs=6))
        pe_pool = ctx.enter_context(tc.tile_pool(name="pe", bufs=6))
        out_pool = ctx.enter_context(tc.tile_pool(name="out", bufs=6))
        psum_pool = ctx.enter_context(tc.tile_pool(name="psum", bufs=4, space="PSUM"))

        zero_t = const_pool.tile([128, 1], F32, tag="zero")
        nc.vector.memset(zero_t[:], 0.0)
        hpi_t = const_pool.tile([128, 1], F32, tag="hpi")
        nc.vector.memset(hpi_t[:], HALF_PI)

        seq_t = const_pool.tile([128, VT * B * 3], F32, tag="seq")
        nc.sync.dma_start(seq_t[:], seq_d[:])
        dv_t = const_pool.tile([128, EH], F32, tag="dv")
        nc.sync.dma_start(dv_t[:], dv_d[:])
        dv2_t = const_pool.tile([128, 2 * KLO], F32, tag="dv2")
        nc.sync.dma_start(dv2_t[:], dv2_d[:])
        eye_t = const_pool.tile([128, 128], F32R, tag="eye")
        nc.sync.dma_start(eye_t[:], eye_d[:])

        for vt in range(VT):
            fw_t = tab_pool.tile([128, E], F32R, tag="fw")
            nc.sync.dma_start(fw_t[:], fw_d[vt * 128 : (vt + 1) * 128, :])
            tw_t = tab_pool.tile([128, E], F32R, tag="tw")
            nc.sync.dma_start(tw_t[:], tw_d[vt * 128 : (vt + 1) * 128, :])
            bs_t = tab_pool.tile([128, E], F32R, tag="bs")
            nc.sync.dma_start(bs_t[:], bs_d[vt * 128 : (vt + 1) * 128, :])
            cmb_t = tab_pool.tile([128, B * 2 * KLO], BF16, tag="cmb")
            nc.sync.dma_start(
                cmb_t[:], cmb_d[:, vt * B * 2 * KLO : (vt + 1) * B * 2 * KLO]
            )

            GB = 4  # batches per pe group (amortizes ACT per-op overhead)
            KHI = EH - KLO  # 336 direct sin lanes
            for g in range(B // GB):
                bs4 = range(g * GB, (g + 1) * GB)

                # group staging: pre-scaled hi angles (GPSIMD) + lo codes (DVE)
                ang4 = ang_pool.tile([128, GB * KHI], F32, tag="ang4")
                r4 = ang_pool.tile([128, GB * 2 * KLO], F32, tag="r4")
                # pe group layout: per b, [ sin(0:384) | cos(384:768) ]
                pe4 = pe_pool.tile([128, GB * E], F32, tag="pe4")
                for i, b in enumerate(bs4):
                    col = vt * B * 3 + b * 3
                    s1 = seq_t[:, col + 1 : col + 2]
                    nc.gpsimd.tensor_tensor(
                        ang4[:, i * KHI : (i + 1) * KHI],
                        dv_t[:, KLO:EH],
                        s1.broadcast_to((128, KHI)),
                        Alu.mult,
                    )
                    nc.vector.scalar_tensor_tensor(
                        r4[:, i * 2 * KLO : (i + 1) * 2 * KLO],
                        dv2_t[:],
                        s1,
                        cmb_t[:, b * 2 * KLO : (b + 1) * 2 * KLO],
                        Alu.mult,
                        Alu.add,
                    )

                # batched Sin ops covering the whole group
                nc.scalar.activation(
                    pe4[:].rearrange("p (i e) -> p i e", i=GB)[:, :, KLO:EH],
                    ang4[:].rearrange("p (i k) -> p i k", i=GB),
                    mybir.ActivationFunctionType.Sin,
                    bias=zero_t[:],
                    scale=1.0,
                )
                nc.scalar.activation(
                    pe4[:].rearrange("p (i e) -> p i e", i=GB)[:, :, EH + KLO : E],
                    ang4[:].rearrange("p (i k) -> p i k", i=GB),
                    mybir.ActivationFunctionType.Sin,
                    bias=hpi_t[:],
                    scale=1.0,
                )
                # lo block: first 48 -> sin half start, next 48 -> cos half start
                nc.scalar.activation(
                    pe4[:]
                    .rearrange("p (i h q) -> p i h q", i=GB, h=2)[:, :, :, 0:KLO],
                    r4[:].rearrange("p (i h q) -> p i h q", i=GB, h=2),
                    mybir.ActivationFunctionType.Sin,
                    bias=zero_t[:],
                    scale=HALF_PI * SIN_SAFETY,
                )

                for i, b in enumerate(bs4):
                    col = vt * B * 3 + b * 3
                    s0 = seq_t[:, col : col + 1]
                    s2 = seq_t[:, col + 2 : col + 3]

                    # diag builds: d0 on ScalarE (Copy with per-row scale),
                    # d2 on GPSIMD - DVE keeps only the merge + lo codes
                    d0 = diag_pool.tile([128, 128], F32R, tag="d0")
                    nc.scalar.mul(d0[:], eye_t[:], s0)
                    d2 = diag_pool.tile([128, 128], F32R, tag="d2")
                    nc.gpsimd.tensor_tensor(
                        d2[:],
                        eye_t[:],
                        s2.broadcast_to((128, 128)).bitcast(F32R),
                        Alu.mult,
                    )

                    # psum = diag(s0)@fw + diag(s2)@tw + I@bsum, split 512/256
                    # to keep each matmul inside one PSUM bank
                    ps = psum_pool.tile([128, E], F32, tag="ps")
                    A, Bx = (0, 512), (512, E)
                    for w, t in ((d0[:], fw_t), (d2[:], tw_t)):
                        for lo, hi in (A, Bx):
                            nc.tensor.matmul(
                                ps[:, lo:hi],
                                w,
                                t[:, lo:hi],
                                start=t is fw_t,
                                stop=False,
                            )
                    for lo, hi in (A, Bx):
                        nc.tensor.matmul(
                            ps[:, lo:hi],
                            eye_t[:],
                            bs_t[:, lo:hi],
                            start=False,
                            stop=True,
                        )

                    # single merge; interleaves sin/cos via the read pattern
                    o_t = out_pool.tile([128, E], F32, tag="o")
                    nc.vector.tensor_add(
                        o_t[:].rearrange("p (q j) -> p q j", j=2),
                        ps[:].rearrange("p (q j) -> p q j", j=2),
                        pe4[:, i * E : (i + 1) * E].rearrange(
                            "p (j q) -> p q j", j=2
                        ),
                    )

                    nc.sync.dma_start(
                        out_d[b, vt * 128 : (vt + 1) * 128, :], o_t[:]
                    )

    nc.finalize()
    return nc


_NC_CACHE: list = []


def _get_nc():
    if not _NC_CACHE:
        _NC_CACHE.append(build_bass())
    return _NC_CACHE[0]


def make_in_maps(sequence, flux_w, flux_b, time_w, time_b):
    import ml_dtypes

    sequence = np.asarray(sequence, dtype=np.float32)
    flux_w = np.asarray(flux_w, dtype=np.float32)
    time_w = np.asarray(time_w, dtype=np.float32)
    bsum = np.asarray(flux_b, dtype=np.float32) + np.asarray(time_b, dtype=np.float32)

    s1_all = sequence[:, :, 1]
    assert np.abs(s1_all).max() < S1_LIMIT, (
        f"positional channel exceeds direct-Sin range: {np.abs(s1_all).max():.3f} "
        f">= {S1_LIMIT:.3f}; raise KLO"
    )

    div = np.exp(
        np.arange(0, E, 2, dtype=np.float32) * np.float32(-math.log(10000.0) / E)
    ).astype(np.float32)
    dv_rep = np.ascontiguousarray(np.broadcast_to(div, (128, EH)))
    # lo block: [48 sin lanes | 48 cos lanes], scaled by 2/pi
    dv2p = (np.concatenate([div[:KLO], div[:KLO]]) * np.float32(2.0 / math.pi)).astype(
        np.float32
    )
    dv2_lo = np.ascontiguousarray(np.broadcast_to(dv2p, (128, 2 * KLO)))
    eye = np.eye(128, dtype=np.float32)

    # combo_n[b,v,h*KLO+k] = j - 4*rint((s1*d_k + j*pi/2)/2pi), j = h (0=sin,1=cos)
    jj = np.concatenate([np.zeros(KLO, np.float64), np.ones(KLO, np.float64)])
    dd = np.concatenate([div[:KLO], div[:KLO]]).astype(np.float64)
    ang = s1_all[:, :, None].astype(np.float64) * dd[None, None, :] + jj * (
        math.pi / 2.0
    )
    n = np.rint(ang / TWO_PI)
    combo_n = (jj[None, None, :] - 4.0 * n).astype(np.float32)
    assert np.abs(combo_n).max() <= 16, "combo codes exceed bf16-exact range"
    combo_bf = combo_n.astype(ml_dtypes.bfloat16)  # small ints: bf16-exact

    in_maps = []
    for c in range(N_CORES):
        v0, v1 = c * V_SHARD, (c + 1) * V_SHARD
        # [B, 512, 3] -> [128p, vt*B*3 + b*3 + ch]
        s = sequence[:, v0:v1, :].reshape(B, VT, 128, 3)
        seq_r = np.ascontiguousarray(s.transpose(2, 1, 0, 3)).reshape(128, VT * B * 3)
        # combo [B, 512, 2*KLO] -> [128p, (vt*B + b)*2*KLO + lane]
        cmb = combo_bf[:, v0:v1, :].reshape(B, VT, 128, 2 * KLO)
        cmb_r = np.ascontiguousarray(cmb.transpose(2, 1, 0, 3)).reshape(
            128, VT * B * 2 * KLO
        )
        in_maps.append(
            {
                "seq": seq_r,
                "fw": np.ascontiguousarray(flux_w[v0:v1]),
                "tw": np.ascontiguousarray(time_w[v0:v1]),
                "bs": np.ascontiguousarray(bsum[v0:v1]),
                "dv": dv_rep,
                "dv2lo": dv2_lo,
                "combo": cmb_r,
                "eye": eye,
            }
        )
    return in_maps


def run(in_maps, trace: bool = False):
    nc = _get_nc()
    return run_bass_kernel_spmd(nc, in_maps, list(range(N_CORES)), trace=trace)


def kernel(sequence, flux_w, flux_b, time_w, time_b) -> np.ndarray:
    in_maps = make_in_maps(sequence, flux_w, flux_b, time_w, time_b)
    res = run(in_maps)
    out = np.concatenate([res.results[c]["out"] for c in range(N_CORES)], axis=1)
    return np.ascontiguousarray(out.astype(np.float32, copy=False))



# revision 11
# speedup vs baseline: 1.1063x; 1.1063x over previous
"""Trainium2 Bass kernel for nn_BERTEmbedding (fused per-index affine + sinusoidal PE).

Math (per batch b, vocab-position v, embed index e):
    out[b,v,e] = s0[b,v]*flux_w[v,e] + flux_b[v,e]
               + s2[b,v]*time_w[v,e] + time_b[v,e]
               + (e even: sin(s1[b,v]*div[e/2]) ; e odd: cos(s1[b,v]*div[(e-1)/2]))

Sharding: vocab axis V=4096 split across 8 cores (512 rows each); every core
handles all 16 batches of its vocab shard.  Weight tables are sharded with the
vocab axis.

Device strategy (per core, 4 v-tiles x 16 batches = 64 work items of [128,768]):
  - Tables fw/tw/bsum shipped bf16 (halves table DMA; bf16 matmuls are
    1 cycle/row at any moving width).
  - TensorE: psum = diag(s0) @ fw + diag(s2) @ tw + I @ bsum, bf16 weights,
    f32 PSUM accumulate, 512/256 column splits per PSUM bank.
  - Diag tiles for a whole v-tile (16 batches x channel) built in ONE batched
    DVE op: D[p, b*128+q] = eye[p,q] * s_ch[p,b] (broadcast APs).
  - ScalarE: pe tile via Sin activation in groups of GB=8 batches, laid out as
    [sin half | cos half].  ScalarE Sin valid on [-pi,pi]:
      k >= KLO: ang = s1*d_k staged on GPSIMD (grouped broadcast mult),
                sin via bias=0, cos via bias=pi/2.
      k <  KLO: host ships integer phase codes combo_n (bf16-exact);
                r' = s1*dv2' + combo on GPSIMD stt, then Sin(scale=pi/2).
  - VectorE: single merge out = psum + pe per item (interleaves sin/cos via
    read APs, evacuates PSUM).  A tunable slice of merges runs on GPSIMD to
    balance engine load.
  - DMA: loads on the ACT HWDGE queue, stores on the SP queue (no
    head-of-line blocking between them); one 393KB store per work item.
"""

import math

import numpy as np

try:
    import concourse.bass as bass
except ImportError:  # harness containers keep the repo at /opt/trn_rl_repo
    import sys

    sys.path.insert(0, "/opt/trn_rl_repo")
    import concourse.bass as bass

import concourse.bacc as bacc
import concourse.tile as tile
from concourse import mybir
from concourse.bass_utils import run_bass_kernel_spmd

B, V, E = 16, 4096, 768
EH = E // 2  # 384 angle lanes
KLO = 48  # angle lanes fixed up via the host combo tensor
KHI = EH - KLO  # 336 direct-sin lanes
N_CORES = 8
V_SHARD = V // N_CORES  # 512
VT = V_SHARD // 128  # 4 v-tiles per core
GB = 8  # batches per pe/sin group
F32 = mybir.dt.float32
BF16 = mybir.dt.bfloat16

TWO_PI = 2.0 * math.pi
HALF_PI = float(np.float32(math.pi / 2.0))
# keep reduced angles strictly inside ScalarE's [-pi, pi] spline domain
SIN_SAFETY = 1.0 - 1e-6
# direct-Sin lanes need |s1|*d_KLO + pi/2 <= pi
S1_LIMIT = (math.pi / 2.0) / math.exp(-KLO * math.log(10000.0) / EH)




def build_bass() -> "bass.Bass":
    from contextlib import ExitStack

    nc = bacc.Bacc(
        "TRN2",
        target_bir_lowering=False,
        debug=False,
        num_devices=N_CORES,
    )
    Alu = mybir.AluOpType

    # dv_ext = [ dv_hi (336 lanes) | dv2' (96 lo lanes scaled 2/pi) ]
    KX = KHI + 2 * KLO  # 432
    seq_d = nc.dram_tensor("seq", [128, VT * B * 3], F32, kind="ExternalInput")
    fw_d = nc.dram_tensor("fw", [V_SHARD, E], BF16, kind="ExternalInput")
    tw_d = nc.dram_tensor("tw", [V_SHARD, E], BF16, kind="ExternalInput")
    bs_d = nc.dram_tensor("bs", [V_SHARD, E], BF16, kind="ExternalInput")
    dv_d = nc.dram_tensor("dv", [128, KX], F32, kind="ExternalInput")
    cmb_d = nc.dram_tensor("combo", [128, VT * B * 2 * KLO], BF16, kind="ExternalInput")
    eye_d = nc.dram_tensor("eye", [128, 128], BF16, kind="ExternalInput")
    out_d = nc.dram_tensor("out", [B, V_SHARD, E], F32, kind="ExternalOutput")

    with tile.TileContext(nc) as tc, ExitStack() as ctx:
        const_pool = ctx.enter_context(tc.tile_pool(name="const", bufs=1))
        tab_pool = ctx.enter_context(tc.tile_pool(name="tables", bufs=2))
        dmat_pool = ctx.enter_context(tc.tile_pool(name="dmat", bufs=2))
        ang_pool = ctx.enter_context(tc.tile_pool(name="ang", bufs=3))
        pe_pool = ctx.enter_context(tc.tile_pool(name="pe", bufs=2))
        out_pool = ctx.enter_context(tc.tile_pool(name="out", bufs=10))
        psum_pool = ctx.enter_context(tc.tile_pool(name="psum", bufs=4, space="PSUM"))

        zero_t = const_pool.tile([128, 1], F32, tag="zero")
        nc.vector.memset(zero_t[:], 0.0)
        hpi_t = const_pool.tile([128, 1], F32, tag="hpi")
        nc.vector.memset(hpi_t[:], HALF_PI)

        seq_t = const_pool.tile([128, VT * B * 3], F32, tag="seq")
        nc.scalar.dma_start(seq_t[:], seq_d[:])
        dv_t = const_pool.tile([128, KX], F32, tag="dv")
        nc.scalar.dma_start(dv_t[:], dv_d[:])
        eye_t = const_pool.tile([128, 128], BF16, tag="eye")
        nc.scalar.dma_start(eye_t[:], eye_d[:])

        item_idx = 0
        for vt in range(VT):
            fw_t = tab_pool.tile([128, E], BF16, tag="fw")
            nc.scalar.dma_start(fw_t[:], fw_d[vt * 128 : (vt + 1) * 128, :])
            tw_t = tab_pool.tile([128, E], BF16, tag="tw")
            nc.scalar.dma_start(tw_t[:], tw_d[vt * 128 : (vt + 1) * 128, :])
            bs_t = tab_pool.tile([128, E], BF16, tag="bs")
            nc.scalar.dma_start(bs_t[:], bs_d[vt * 128 : (vt + 1) * 128, :])
            cmb_t = tab_pool.tile([128, B * 2 * KLO], BF16, tag="cmb")
            nc.scalar.dma_start(
                cmb_t[:], cmb_d[:, vt * B * 2 * KLO : (vt + 1) * B * 2 * KLO]
            )

            # batched diag builds: D[p, b*128+q] = eye[p,q] * s_ch[p,b]
            # one DVE op per channel covering all 16 batches of this v-tile
            seq_vt = seq_t[:, vt * B * 3 : (vt + 1) * B * 3].rearrange(
                "p (b c) -> p b c", c=3
            )
            dmats = {}
            for ch, eng in ((0, nc.vector), (2, nc.gpsimd)):
                d_t = dmat_pool.tile([128, B * 128], BF16, tag=f"d{ch}")
                eye_b = eye_t[:].unsqueeze(1).broadcast_to((128, B, 128))
                s_b = seq_vt[:, :, ch : ch + 1].broadcast_to((128, B, 128))
                eng.tensor_tensor(
                    d_t[:].rearrange("p (b q) -> p b q", b=B),
                    eye_b,
                    s_b,
                    Alu.mult,
                )
                dmats[ch] = d_t

            for g in range(B // GB):
                bs8 = range(g * GB, (g + 1) * GB)

                # group staging: ang_ext = s1 * [dv_hi | dv2'] on GPSIMD
                # (two 4-batch broadcast mults), then lo codes
                # r' = ang_ext[lo] + combo in one grouped DVE add
                ang_g = ang_pool.tile([128, GB * KX], F32, tag="ang")
                r4_g = ang_pool.tile([128, GB * 2 * KLO], F32, tag="r4")
                ang3 = ang_g[:].rearrange("p (i k) -> p i k", i=GB)
                for h in range(2):
                    i0 = h * (GB // 2)
                    b0 = g * GB + i0
                    s1_b = (
                        seq_vt[:, b0 : b0 + GB // 2, 1:2]
                        .broadcast_to((128, GB // 2, KX))
                    )
                    dv_b = dv_t[:].unsqueeze(1).broadcast_to((128, GB // 2, KX))
                    nc.gpsimd.tensor_tensor(
                        ang3[:, i0 : i0 + GB // 2, :],
                        dv_b,
                        s1_b,
                        Alu.mult,
                    )
                nc.vector.tensor_tensor(
                    r4_g[:].rearrange("p (i l) -> p i l", i=GB),
                    ang3[:, :, KHI:KX],
                    cmb_t[:, g * GB * 2 * KLO : (g + 1) * GB * 2 * KLO].rearrange(
                        "p (i l) -> p i l", i=GB
                    ),
                    Alu.add,
                )

                # pe group layout per batch: [ sin(0:384) | cos(384:768) ]
                pe_g = pe_pool.tile([128, GB * E], F32, tag="pe")
                nc.scalar.activation(
                    pe_g[:].rearrange("p (i e) -> p i e", i=GB)[:, :, KLO:EH],
                    ang3[:, :, 0:KHI],
                    mybir.ActivationFunctionType.Sin,
                    bias=zero_t[:],
                    scale=1.0,
                )
                nc.scalar.activation(
                    pe_g[:].rearrange("p (i e) -> p i e", i=GB)[:, :, EH + KLO : E],
                    ang3[:, :, 0:KHI],
                    mybir.ActivationFunctionType.Sin,
                    bias=hpi_t[:],
                    scale=1.0,
                )
                # lo block: first 48 -> sin half start, next 48 -> cos half start
                nc.scalar.activation(
                    pe_g[:].rearrange("p (i h q) -> p i h q", i=GB, h=2)[
                        :, :, :, 0:KLO
                    ],
                    r4_g[:].rearrange("p (i h q) -> p i h q", i=GB, h=2),
                    mybir.ActivationFunctionType.Sin,
                    bias=zero_t[:],
                    scale=HALF_PI * SIN_SAFETY,
                )

                for i, b in enumerate(bs8):
                    # psum = diag(s0)@fw + diag(s2)@tw + I@bsum, split 512/256
                    # to keep each matmul inside one PSUM bank
                    ps = psum_pool.tile([128, E], F32, tag="ps")
                    A, Bx = (0, 512), (512, E)
                    for lo, hi in (A, Bx):
                        nc.tensor.matmul(
                            ps[:, lo:hi],
                            dmats[0][:, b * 128 : (b + 1) * 128],
                            fw_t[:, lo:hi],
                            start=True,
                            stop=False,
                        )
                        nc.tensor.matmul(
                            ps[:, lo:hi],
                            dmats[2][:, b * 128 : (b + 1) * 128],
                            tw_t[:, lo:hi],
                            start=False,
                            stop=False,
                        )
                        nc.tensor.matmul(
                            ps[:, lo:hi],
                            eye_t[:],
                            bs_t[:, lo:hi],
                            start=False,
                            stop=True,
                        )

                    # single merge; interleaves sin/cos via the read pattern
                    o_t = out_pool.tile([128, E], F32, tag="o")
                    nc.vector.tensor_tensor(
                        o_t[:].rearrange("p (q j) -> p q j", j=2),
                        ps[:].rearrange("p (q j) -> p q j", j=2),
                        pe_g[:, i * E : (i + 1) * E].rearrange(
                            "p (j q) -> p q j", j=2
                        ),
                        Alu.add,
                    )

                    nc.sync.dma_start(
                        out_d[b, vt * 128 : (vt + 1) * 128, :], o_t[:]
                    )
                    item_idx += 1

    nc.finalize()
    return nc


_NC_CACHE: list = []


def _get_nc():
    if not _NC_CACHE:
        _NC_CACHE.append(build_bass())
    return _NC_CACHE[0]


def make_in_maps(sequence, flux_w, flux_b, time_w, time_b):
    import ml_dtypes

    sequence = np.asarray(sequence, dtype=np.float32)
    flux_w = np.asarray(flux_w, dtype=np.float32)
    time_w = np.asarray(time_w, dtype=np.float32)
    bsum = np.asarray(flux_b, dtype=np.float32) + np.asarray(time_b, dtype=np.float32)

    s1_all = sequence[:, :, 1]
    assert np.abs(s1_all).max() < S1_LIMIT, (
        f"positional channel exceeds direct-Sin range: {np.abs(s1_all).max():.3f} "
        f">= {S1_LIMIT:.3f}; raise KLO"
    )

    div = np.exp(
        np.arange(0, E, 2, dtype=np.float32) * np.float32(-math.log(10000.0) / E)
    ).astype(np.float32)
    # dv_ext = [ dv_hi | 48 lo sin lanes * 2/pi | 48 lo cos lanes * 2/pi ]
    dv2p = (np.concatenate([div[:KLO], div[:KLO]]) * np.float32(2.0 / math.pi)).astype(
        np.float32
    )
    dv_ext = np.concatenate([div[KLO:], dv2p]).astype(np.float32)
    dv_rep = np.ascontiguousarray(np.broadcast_to(dv_ext, (128, KHI + 2 * KLO)))
    eye_bf = np.eye(128, dtype=np.float32).astype(ml_dtypes.bfloat16)

    # combo_n[b,v,h*KLO+k] = j - 4*rint((s1*d_k + j*pi/2)/2pi), j = h (0=sin,1=cos)
    jj = np.concatenate([np.zeros(KLO, np.float64), np.ones(KLO, np.float64)])
    dd = np.concatenate([div[:KLO], div[:KLO]]).astype(np.float64)
    ang = s1_all[:, :, None].astype(np.float64) * dd[None, None, :] + jj * (
        math.pi / 2.0
    )
    n = np.rint(ang / TWO_PI)
    combo_n = (jj[None, None, :] - 4.0 * n).astype(np.float32)
    assert np.abs(combo_n).max() <= 16, "combo codes exceed bf16-exact range"
    combo_bf = combo_n.astype(ml_dtypes.bfloat16)  # small ints: bf16-exact

    fw_bf = flux_w.astype(ml_dtypes.bfloat16)
    tw_bf = time_w.astype(ml_dtypes.bfloat16)
    bs_bf = bsum.astype(ml_dtypes.bfloat16)

    in_maps = []
    for c in range(N_CORES):
        v0, v1 = c * V_SHARD, (c + 1) * V_SHARD
        # [B, 512, 3] -> [128p, vt*B*3 + b*3 + ch]
        s = sequence[:, v0:v1, :].reshape(B, VT, 128, 3)
        seq_r = np.ascontiguousarray(s.transpose(2, 1, 0, 3)).reshape(128, VT * B * 3)
        # combo [B, 512, 2*KLO] -> [128p, (vt*B + b)*2*KLO + lane]
        cmb = combo_bf[:, v0:v1, :].reshape(B, VT, 128, 2 * KLO)
        cmb_r = np.ascontiguousarray(cmb.transpose(2, 1, 0, 3)).reshape(
            128, VT * B * 2 * KLO
        )
        in_maps.append(
            {
                "seq": seq_r,
                "fw": np.ascontiguousarray(fw_bf[v0:v1]),
                "tw": np.ascontiguousarray(tw_bf[v0:v1]),
                "bs": np.ascontiguousarray(bs_bf[v0:v1]),
                "dv": dv_rep,
                "combo": cmb_r,
                "eye": eye_bf,
            }
        )
    return in_maps


def run(in_maps, trace: bool = False):
    nc = _get_nc()
    return run_bass_kernel_spmd(nc, in_maps, list(range(N_CORES)), trace=trace)


def kernel(sequence, flux_w, flux_b, time_w, time_b) -> np.ndarray:
    in_maps = make_in_maps(sequence, flux_w, flux_b, time_w, time_b)
    res = run(in_maps)
    out = np.concatenate([res.results[c]["out"] for c in range(N_CORES)], axis=1)
    return np.ascontiguousarray(out.astype(np.float32, copy=False))
